# revision 27
# baseline (speedup 1.0000x reference)
"""Multi-head attention (B=2, S=2048, D=1024, H=16, causal mask) on 8 TRN2 cores.

Sharding: core c handles batch b = c//4 and 4 heads g = c%4 (dims 256g..256g+256
of the projection space).  Each core computes a partial output [S, D] (its 4
heads' contribution to the out-projection); the host sums the 4 partials per
batch and adds the output bias.

Device layout (per core) keeps the sequence axis on the SBUF free dimension:
  QT, KT  [256, 2048]  (head-dim on partitions, 2 head-pairs of 128)
  V_aug   16 tiles [128, 4, 65]  (seq on partitions; per head 64 dims + ones col)
  scores  S.T tiles [128 k, 512 q] per head; causal blocks above diagonal skipped
  exp     ScalarE, scale=1/8; causal diag block masked post-exp by a 0/1 bf16
          tri multiply on DVE (cheaper than the f32 PSUM bias add)
  ctx.T   [65, 512] PSUM per (head, q-chunk); row 64 = softmax denominator l
  norm    l copied to SBUF (reciprocal_approx_fast needs IEEE fp32 bits,
          so no direct PSUM read), reciprocal_approx_fast,
          partition_broadcast, DVE multiply
  out     ctxT (4 heads stacked, [256, 2048]) @ o_w slice -> [2048, 1024]

Perf structure:
  - all-bf16 compute: fp8 (even on the q/k path) fails the 2e-2 gate —
    multiplicative quantization noise propagates to the output at full
    relative strength (~5%); bf16 gives ~0.6%.
  - input DMAs are emitted in strict need-order, round-robined across the
    three DMA queues (sync/scalar HWDGE + gpsimd SWDGE): within a queue
    triggers process in order and the HW engines round-robin across
    queues, so aggregate HBM bandwidth always serves the next-needed
    tensors instead of fair-sharing with stage-2/3 prefetch.  Out DMA is
    one [128,1024] per 128-row block on sync.
  - the out-projection of q-chunks 2,3 is deferred to the end of the
    program: the last q-chunk's attention is exp(ACT)-throughput-bound,
    and those matmuls are the PE work that hides it.
"""

import numpy as np
from contextlib import ExitStack

import concourse.bacc as bacc
import concourse.bass as bass
import concourse.tile as tile
from concourse import mybir
from concourse.hw_specs import TRN2Spec

# Schedule against the sustained PE clock: warm matmuls run at 2.4 GHz but
# cold-HAM stretches and P0 power dips put the average nearer 2.2; the
# frozen per-engine orders match hardware better with the conservative
# number.  Affects only scheduling, not correctness.
TRN2Spec.PE_CYCLE = 1e9 / 2.2e9



P = 128
S = 2048
D = 1024
N_HEADS_TOT = 16
HEADS = 4            # per core
HD = 64
M_DIM = HEADS * HD   # 256
KC = 8               # embed-dim 128-chunks (bf16 v path)
QCW = 512            # q chunk width
NQC = S // QCW       # 4
NKT = S // P         # 16 k-tiles
F32 = mybir.dt.float32
BF16 = mybir.dt.bfloat16
EXPF = mybir.ActivationFunctionType.Exp
NEG = -1.0e9

TRACE = False
LAST_RESULTS = None
_NC_CACHE = {}


def build_nc(mode: str, compile_: bool = True,
             has_bias: bool = False) -> bass.Bass:
    """mode in {causal, nomask, generic}"""
    nc = bacc.Bacc("TRN2", target_bir_lowering=False, debug=False)
    # packed bf16 x: [p, stage, kc, c] ; d = 128*kc + p, s = 512*stage + c
    xin = {}
    for nm in ("q", "k", "v"):
        xin[nm] = nc.dram_tensor(f"x{nm}16", [P, NQC * KC * QCW], BF16,
                                 kind="ExternalInput").ap()
    # packed bf16 weights: [p, kc, m] ; row d = 128*kc + p
    win = {}
    for nm in ("q", "k", "v"):
        win[nm] = nc.dram_tensor(f"w{nm}16", [P, KC * M_DIM], BF16,
                                 kind="ExternalInput").ap()
    augs = {}
    if has_bias:
        for nm in ("wqA", "wkA", "wvA"):
            augs[nm] = nc.dram_tensor(nm, [1, M_DIM], BF16,
                                      kind="ExternalInput").ap()
    ow = nc.dram_tensor("owT", [M_DIM, D], BF16, kind="ExternalInput").ap()
    btri = nc.dram_tensor("btri", [P, P], BF16, kind="ExternalInput").ap()
    bfull = None
    if mode == "generic":
        bfull = nc.dram_tensor("biasT", [S, S], F32, kind="ExternalInput").ap()
    out = nc.dram_tensor("out", [S, D], BF16, kind="ExternalOutput").ap()

    with tile.TileContext(nc) as tc, ExitStack() as ctx:
        consts = ctx.enter_context(tc.tile_pool(name="consts", bufs=1))
        xpool = ctx.enter_context(tc.tile_pool(name="xpool", bufs=1))
        qkv = ctx.enter_context(tc.tile_pool(name="qkv", bufs=1))
        ppool = ctx.enter_context(tc.tile_pool(name="ppool", bufs=8))
        bpool = ctx.enter_context(tc.tile_pool(name="bpool", bufs=2))
        small = ctx.enter_context(tc.tile_pool(name="small", bufs=4))
        outp = ctx.enter_context(tc.tile_pool(name="outp", bufs=3))
        spool = ctx.enter_context(tc.tile_pool(name="spsum", bufs=2, space="PSUM"))
        opool = ctx.enter_context(tc.tile_pool(name="opsum", bufs=2, space="PSUM"))
        cpool = ctx.enter_context(tc.tile_pool(name="cpsum", bufs=2, space="PSUM"))

        # ---- weights + x tiles: contiguous DMAs, ordered by first need.
        # Strict need-order, round-robined across the three DMA queues:
        # within a queue triggers are processed in order, and the HW engines
        # round-robin across queues, so global need-order emission keeps the
        # aggregate HBM bandwidth on the next-needed tensors.
        rrq = [nc.sync, nc.scalar, nc.gpsimd]
        rr_i = [0]

        def next_q():
            e = rrq[rr_i[0] % 3]
            rr_i[0] += 1
            return e

        w_sb = {}
        for key in ("q", "k", "v"):
            t = consts.tile([P, KC, M_DIM], BF16, name=f"w16{key}")
            next_q().dma_start(out=t, in_=win[key].rearrange(
                "p (kc m) -> p kc m", kc=KC))
            w_sb[key] = t

        # xt[key][stage] = list of (tile, kc0, nkc) pieces
        xt = {"q": [], "k": [], "v": []}
        for key in ("q", "k", "v"):
            for n in range(NQC):
                xt[key].append([])

        def load_x(key, n, halves):
            dst = xt[key][n]
            nh = 2 if halves else 1
            nkc = KC // nh
            for h in range(nh):
                t = xpool.tile([P, nkc, QCW], BF16, name=f"x{key}{n}{h}")
                c0 = (KC * n + nkc * h) * QCW
                next_q().dma_start(
                    out=t,
                    in_=xin[key][:, c0:c0 + nkc * QCW].rearrange(
                        "p (kc c) -> p kc c", kc=nkc))
                dst.append((t, nkc * h, nkc))

        load_x("q", 0, True)
        load_x("k", 0, True)
        load_x("v", 0, True)
        btri_sb = consts.tile([P, P], BF16, name="btri_sb")
        next_q().dma_start(out=btri_sb, in_=btri)
        for n in range(1, NQC):
            for key in ("q", "k", "v"):
                load_x(key, n, False)
        ow_sb = []
        for pr in range(2):
            t = consts.tile([P, D], BF16, name=f"ow{pr}")
            next_q().dma_start(out=t, in_=ow[P * pr:P * (pr + 1), :])
            ow_sb.append(t)

        # PE warm-up: dummy matmuls with no DMA deps keep the PE busy through
        # the preamble/DMA-ramp window so the HAM clock-gate opens before the
        # first real matmul arrives.
        warm = consts.tile([P, QCW], BF16, name="warm")
        nc.vector.memset(warm, 0.0)

        def pe_filler(count, tag):
            wps = spool.tile([P, 2, QCW], F32, name="s_ps")
            for i in range(count):
                nc.tensor.matmul(
                    wps[:, 0, :], lhsT=warm[:, 0:P], rhs=warm,
                    start=(i == 0), stop=(i == count - 1))

        pe_filler(18, "head")

        w_aug = {}
        ones_row = None
        if has_bias:
            ones_row = consts.tile([1, QCW], BF16, name="ones_row")
            nc.vector.memset(ones_row, 1.0)
            for key, nm in (("q", "wqA"), ("k", "wkA"), ("v", "wvA")):
                t = consts.tile([1, M_DIM], BF16, name=nm)
                nc.scalar.dma_start(out=t, in_=augs[nm])
                w_aug[key] = t

        QT = [qkv.tile([P, S], BF16, name=f"QT{pr}") for pr in range(2)]
        KT = [qkv.tile([P, S], BF16, name=f"KT{pr}") for pr in range(2)]
        CT = [qkv.tile([P, S], BF16, name=f"CT{pr}") for pr in range(2)]
        VA = [qkv.tile([P, HEADS, HD + 1], BF16, name=f"VA{t}") for t in range(NKT)]
        # softmax-denominator ones column, written once
        for t in range(NKT):
            nc.gpsimd.memset(VA[t][:, :, HD:HD + 1], 1.0)


        def emit_outproj(qc):
            for mq in range(QCW // P):
                out_sb = outp.tile([P, D], BF16, name="out_sb")
                q0 = QCW * qc + P * mq
                for ne in range(2):
                    o_ps = opool.tile([P, QCW], F32, name="o_ps")
                    for pr2 in range(2):
                        nc.tensor.matmul(
                            o_ps,
                            lhsT=CT[pr2][:, q0:q0 + P],
                            rhs=ow_sb[pr2][:, QCW * ne:QCW * (ne + 1)],
                            start=(pr2 == 0), stop=(pr2 == 1))
                    nc.vector.tensor_copy(out_sb[:, QCW * ne:QCW * (ne + 1)], o_ps)
                nc.sync.dma_start(out=out[q0:q0 + P, :], in_=out_sb)

        tri_bc = btri_sb.rearrange("p (a q) -> p a q", a=1).to_broadcast([P, 2, P])

        for n in range(NQC):
            # ---- stage n projections: q/k columns + v rows [512n, 512n+512) ----
            for key, dest in (("q", QT), ("k", KT)):
                for m in range(2):
                    ps = opool.tile([P, QCW], F32, name="o_ps")
                    for pc, (xtile, kc0, nkc) in enumerate(xt[key][n]):
                        last_pc = pc == len(xt[key][n]) - 1
                        for kc in range(nkc):
                            nc.tensor.matmul(
                                ps,
                                lhsT=w_sb[key][:, kc0 + kc, P * m:P * (m + 1)],
                                rhs=xtile[:, kc, :],
                                start=(kc0 + kc == 0),
                                stop=(not has_bias and last_pc and
                                      kc == nkc - 1))
                    if has_bias:
                        nc.tensor.matmul(
                            ps,
                            lhsT=w_aug[key][0:1, P * m:P * (m + 1)],
                            rhs=ones_row,
                            start=False, stop=True)
                    nc.vector.tensor_copy(
                        dest[m][:, QCW * n:QCW * (n + 1)], ps)
            for mv in range(4):
                m = 4 * n + mv
                ps = opool.tile([P, QCW], F32, name="o_ps")
                for pc, (xtile, kc0, nkc) in enumerate(xt["v"][n]):
                    last_pc = pc == len(xt["v"][n]) - 1
                    for kc in range(nkc):
                        nc.tensor.matmul(
                            ps[:, 0:M_DIM],
                            lhsT=xtile[:, kc, P * mv:P * (mv + 1)],
                            rhs=w_sb["v"][:, kc0 + kc, :],
                            start=(kc0 + kc == 0),
                            stop=(not has_bias and last_pc and kc == nkc - 1))
                if has_bias:
                    nc.tensor.matmul(
                        ps[:, 0:M_DIM],
                        lhsT=ones_row[0:1, 0:P],
                        rhs=w_aug["v"],
                        start=False, stop=True)
                nc.vector.tensor_copy(
                    VA[m][:, :, 0:HD],
                    ps[:, 0:M_DIM].rearrange("p (h d) -> p h d", h=HEADS))
            if n >= 2:
                emit_outproj(n - 2)

            # ---- stage n attention (q chunk n) ----
            qc = n
            for pr in range(2):
                nt = 4 * qc + 4 if mode == "causal" else NKT
                ctxs = [cpool.tile([HD + 1, QCW], F32, name="ctx_ps")
                        for _ in range(2)]
                queues = ([], [])

                def flush_ctx(j):
                    t0, p0, o0 = queues[j].pop(0)
                    nc.tensor.matmul(
                        ctxs[j][:, o0:],
                        lhsT=VA[t0][:, 2 * pr + j, :],
                        rhs=p0[:, j, o0:],
                        start=(t0 == 0), stop=(t0 == nt - 1),
                        skip_group_check=True)

                for t in range(nt):
                    o = max(0, P * t - QCW * qc) if mode == "causal" else 0
                    s_ps = spool.tile([P, 2, QCW], F32, name="s_ps")
                    for j in range(2):
                        nc.tensor.matmul(
                            s_ps[:, j, o:],
                            lhsT=KT[pr][HD * j:HD * (j + 1), P * t:P * (t + 1)],
                            rhs=QT[pr][HD * j:HD * (j + 1),
                                       QCW * qc + o:QCW * (qc + 1)],
                            start=True, stop=True,
                            tile_position=(HD * j, 0))
                    if mode == "generic":
                        bt = bpool.tile([P, QCW], F32, name="bt")
                        nc.sync.dma_start(
                            out=bt,
                            in_=bfull[P * t:P * (t + 1), QCW * qc:QCW * (qc + 1)])
                        nc.vector.tensor_add(
                            s_ps, s_ps,
                            bt.rearrange("p (a q) -> p a q", a=1)
                            .to_broadcast([P, 2, QCW]))
                    p_sb = ppool.tile([P, 2, QCW], BF16, name="p_sb")
                    nc.scalar.activation(
                        p_sb[:, :, o:], s_ps[:, :, o:], EXPF, scale=0.125)
                    if mode == "causal" and t >= 4 * qc:
                        # zero the upper triangle of the diagonal block
                        nc.vector.tensor_mul(
                            p_sb[:, :, o:o + P], p_sb[:, :, o:o + P], tri_bc)
                    for j in range(2):
                        queues[j].append((t, p_sb, o))
                    for j in range(2):
                        if len(queues[j]) > 2:
                            flush_ctx(j)
                if n == NQC - 1 and pr == 1:
                    # stage 3 is exp(ACT)-bound at the end: keep the PE busy
                    # (HAM warm) while the last exp tiles drain
                    pe_filler(12, "drain")
                for j in range(2):
                    while queues[j]:
                        flush_ctx(j)
                for j in range(2):
                    ctx_ps = ctxs[j]
                    l_sb = small.tile([1, QCW], F32, name="l_sb", bufs=3)
                    nc.vector.tensor_copy(l_sb, ctx_ps[HD:HD + 1, :])
                    r_sb = small.tile([1, QCW], F32, name="r_sb", bufs=3)
                    nc.vector.reciprocal_approx_fast(out=r_sb, in_=l_sb)
                    rbc = ppool.tile([HD, QCW], F32, name="rbc", bufs=2)
                    nc.gpsimd.partition_broadcast(out_ap=rbc, in_ap=r_sb)
                    nc.vector.tensor_mul(
                        CT[pr][HD * j:HD * (j + 1), QCW * qc:QCW * (qc + 1)],
                        ctx_ps[0:HD, :], rbc)

            if n == NQC - 1:
                # bridge the last norm-chain latency so the PE stays warm
                # (HAM) going into the final out-projection
                pe_filler(10, "tail")
        # out-projection of the last two q-chunks runs at the end: the last
        # q-chunk's attention is exp(ACT)-bound, and these matmuls are the
        # main PE work left to hide that
        emit_outproj(2)
        emit_outproj(3)

    if compile_:
        nc.compile()
    return nc


def _get_nc(mode, has_bias):
    key = (mode, has_bias)
    if key not in _NC_CACHE:
        _NC_CACHE[key] = build_nc(mode, has_bias=has_bias)
    return _NC_CACHE[key]


def _tri01():
    # 1.0 where q(col) >= k(row), else 0 (lower triangle incl diagonal kept)
    g = np.arange(P, dtype=np.int64)
    return np.where(g[None, :] >= g[:, None], 1.0, 0.0).astype(np.float32)


def host_prep(query, key, value, attn_mask, q_w, q_b, k_w, k_b, v_w, v_b, o_w, o_b):
    """Build (mode, in_maps) for the 8 cores."""
    mask = np.asarray(attn_mask).astype(bool)
    if np.array_equal(mask, np.triu(np.ones((S, S), bool), 1)):
        mode = "causal"
    elif not mask.any():
        mode = "nomask"
    else:
        mode = "generic"

    import ml_dtypes
    bf16 = ml_dtypes.bfloat16

    def prep_x(x):
        # [p, n, kc, c]; d = 128*kc + p, s = 512*n + c
        xtr = np.ascontiguousarray(np.asarray(x).T)
        a = xtr.reshape(KC, P, NQC, QCW).transpose(1, 2, 0, 3)
        return np.ascontiguousarray(a).astype(bf16).reshape(P, NQC * KC * QCW)

    xs = {}
    for b in range(2):
        xs[b] = (prep_x(np.asarray(query)[b]),
                 prep_x(np.asarray(key)[b]),
                 prep_x(np.asarray(value)[b]))

    tri = _tri01().astype(bf16)
    biasT = None
    if mode == "generic":
        biasT = np.ascontiguousarray(
            np.where(mask, np.float32(NEG), np.float32(0.0)).T)

    def prep_w(w, sl):
        wt = np.ascontiguousarray(np.asarray(w)[sl].T)  # [1024, 256]
        a = wt.reshape(KC, P, M_DIM).transpose(1, 0, 2)
        return np.ascontiguousarray(a).astype(bf16).reshape(P, KC * M_DIM)

    def prep_aug(bvec, sl):
        return np.ascontiguousarray(
            np.asarray(bvec)[sl][None, :]).astype(bf16)

    has_bias = any(np.asarray(b).any() for b in (q_b, k_b, v_b))
    in_maps = []
    for c in range(8):
        b, g = divmod(c, 4)
        sl = slice(M_DIM * g, M_DIM * (g + 1))
        m = {
            "xq16": xs[b][0], "xk16": xs[b][1], "xv16": xs[b][2],
            "wq16": prep_w(q_w, sl),
            "wk16": prep_w(k_w, sl),
            "wv16": prep_w(v_w, sl),
            "owT": np.ascontiguousarray(np.asarray(o_w)[:, sl].T).astype(bf16),
            "btri": tri,
        }
        if has_bias:
            m["wqA"] = prep_aug(q_b, sl)
            m["wkA"] = prep_aug(k_b, sl)
            m["wvA"] = prep_aug(v_b, sl)
        if mode == "generic":
            m["biasT"] = biasT
        in_maps.append(m)
    return mode, has_bias, in_maps


def kernel(**inputs) -> np.ndarray:
    global LAST_RESULTS
    from concourse.bass_utils import run_bass_kernel_spmd

    mode, has_bias, in_maps = host_prep(**inputs)
    nc = _get_nc(mode, has_bias)
    res = run_bass_kernel_spmd(nc, in_maps, core_ids=list(range(8)), trace=TRACE)
    LAST_RESULTS = res
    parts = [np.asarray(res.results[c]["out"]).astype(np.float32)
             for c in range(8)]
    o_b = np.asarray(inputs["o_b"]).astype(np.float32)
    out = np.stack([
        parts[0] + parts[1] + parts[2] + parts[3],
        parts[4] + parts[5] + parts[6] + parts[7],
    ], axis=0) + o_b[None, None, :]
    return out.astype(np.float32)


# revision 28
# speedup vs baseline: 1.0049x; 1.0049x over previous
"""Multi-head attention (B=2, S=2048, D=1024, H=16, causal mask) on 8 TRN2 cores.

Sharding: core c handles batch b = c//4 and 4 heads g = c%4 (dims 256g..256g+256
of the projection space).  Each core computes a partial output [S, D] (its 4
heads' contribution to the out-projection); the host sums the 4 partials per
batch and adds the output bias.

Device layout (per core) keeps the sequence axis on the SBUF free dimension:
  QT, KT  [256, 2048]  (head-dim on partitions, 2 head-pairs of 128)
  V_aug   16 tiles [128, 4, 65]  (seq on partitions; per head 64 dims + ones col)
  scores  S.T tiles [128 k, 512 q] per head; causal blocks above diagonal skipped
  exp     ScalarE, scale=1/8; causal diag block masked post-exp by a 0/1 bf16
          tri multiply on DVE (cheaper than the f32 PSUM bias add)
  ctx.T   [65, 512] PSUM per (head, q-chunk); row 64 = softmax denominator l
  norm    l copied to SBUF (reciprocal_approx_fast needs IEEE fp32 bits,
          so no direct PSUM read), reciprocal_approx_fast,
          partition_broadcast, DVE multiply
  out     ctxT (4 heads stacked, [256, 2048]) @ o_w slice -> [2048, 1024]

Perf structure:
  - all-bf16 compute: fp8 (even on the q/k path) fails the 2e-2 gate —
    multiplicative quantization noise propagates to the output at full
    relative strength (~5%); bf16 gives ~0.6%.
  - input DMAs are emitted in strict need-order, round-robined across the
    three DMA queues (sync/scalar HWDGE + gpsimd SWDGE): within a queue
    triggers process in order and the HW engines round-robin across
    queues, so aggregate HBM bandwidth always serves the next-needed
    tensors instead of fair-sharing with stage-2/3 prefetch.  Out DMA is
    one [128,1024] per 128-row block on sync.
  - the out-projection of q-chunks 2,3 is deferred to the end of the
    program: the last q-chunk's attention is exp(ACT)-throughput-bound,
    and those matmuls are the PE work that hides it.
"""

import numpy as np
from contextlib import ExitStack

import concourse.bacc as bacc
import concourse.bass as bass
import concourse.tile as tile
from concourse import mybir



P = 128
S = 2048
D = 1024
N_HEADS_TOT = 16
HEADS = 4            # per core
HD = 64
M_DIM = HEADS * HD   # 256
KC = 8               # embed-dim 128-chunks (bf16 v path)
QCW = 512            # q chunk width
NQC = S // QCW       # 4
NKT = S // P         # 16 k-tiles
F32 = mybir.dt.float32
BF16 = mybir.dt.bfloat16
EXPF = mybir.ActivationFunctionType.Exp
NEG = -1.0e9

TRACE = False
LAST_RESULTS = None
_NC_CACHE = {}


def build_nc(mode: str, compile_: bool = True,
             has_bias: bool = False) -> bass.Bass:
    """mode in {causal, nomask, generic}"""
    nc = bacc.Bacc("TRN2", target_bir_lowering=False, debug=False)
    # packed bf16 x: [p, stage, kc, c] ; d = 128*kc + p, s = 512*stage + c
    xin = {}
    for nm in ("q", "k", "v"):
        xin[nm] = nc.dram_tensor(f"x{nm}16", [P, NQC * KC * QCW], BF16,
                                 kind="ExternalInput").ap()
    # packed bf16 weights: [p, kc, m] ; row d = 128*kc + p
    win = {}
    for nm in ("q", "k", "v"):
        win[nm] = nc.dram_tensor(f"w{nm}16", [P, KC * M_DIM], BF16,
                                 kind="ExternalInput").ap()
    augs = {}
    if has_bias:
        for nm in ("wqA", "wkA", "wvA"):
            augs[nm] = nc.dram_tensor(nm, [1, M_DIM], BF16,
                                      kind="ExternalInput").ap()
    ow = nc.dram_tensor("owT", [M_DIM, D], BF16, kind="ExternalInput").ap()
    btri = nc.dram_tensor("btri", [P, P], BF16, kind="ExternalInput").ap()
    bfull = None
    if mode == "generic":
        bfull = nc.dram_tensor("biasT", [S, S], F32, kind="ExternalInput").ap()
    out = nc.dram_tensor("out", [S, D], BF16, kind="ExternalOutput").ap()

    with tile.TileContext(nc) as tc, ExitStack() as ctx:
        consts = ctx.enter_context(tc.tile_pool(name="consts", bufs=1))
        xpool = ctx.enter_context(tc.tile_pool(name="xpool", bufs=1))
        qkv = ctx.enter_context(tc.tile_pool(name="qkv", bufs=1))
        ppool = ctx.enter_context(tc.tile_pool(name="ppool", bufs=8))
        bpool = ctx.enter_context(tc.tile_pool(name="bpool", bufs=2))
        small = ctx.enter_context(tc.tile_pool(name="small", bufs=4))
        outp = ctx.enter_context(tc.tile_pool(name="outp", bufs=3))
        spool = ctx.enter_context(tc.tile_pool(name="spsum", bufs=2, space="PSUM"))
        opool = ctx.enter_context(tc.tile_pool(name="opsum", bufs=2, space="PSUM"))
        cpool = ctx.enter_context(tc.tile_pool(name="cpsum", bufs=2, space="PSUM"))

        # ---- weights + x tiles: contiguous DMAs, ordered by first need.
        # Strict need-order, round-robined across the three DMA queues:
        # within a queue triggers are processed in order, and the HW engines
        # round-robin across queues, so global need-order emission keeps the
        # aggregate HBM bandwidth on the next-needed tensors.
        rrq = [nc.sync, nc.scalar, nc.gpsimd]
        rr_i = [0]

        def next_q():
            e = rrq[rr_i[0] % 3]
            rr_i[0] += 1
            return e

        w_sb = {}
        for key in ("q", "k", "v"):
            t = consts.tile([P, KC, M_DIM], BF16, name=f"w16{key}")
            next_q().dma_start(out=t, in_=win[key].rearrange(
                "p (kc m) -> p kc m", kc=KC))
            w_sb[key] = t

        # xt[key][stage] = list of (tile, kc0, nkc) pieces
        xt = {"q": [], "k": [], "v": []}
        for key in ("q", "k", "v"):
            for n in range(NQC):
                xt[key].append([])

        def load_x(key, n, halves):
            dst = xt[key][n]
            nh = 2 if halves else 1
            nkc = KC // nh
            for h in range(nh):
                t = xpool.tile([P, nkc, QCW], BF16, name=f"x{key}{n}{h}")
                c0 = (KC * n + nkc * h) * QCW
                next_q().dma_start(
                    out=t,
                    in_=xin[key][:, c0:c0 + nkc * QCW].rearrange(
                        "p (kc c) -> p kc c", kc=nkc))
                dst.append((t, nkc * h, nkc))

        load_x("q", 0, True)
        load_x("k", 0, True)
        load_x("v", 0, True)
        btri_sb = consts.tile([P, P], BF16, name="btri_sb")
        next_q().dma_start(out=btri_sb, in_=btri)
        for n in range(1, NQC):
            for key in ("q", "k", "v"):
                load_x(key, n, False)
        ow_sb = []
        for pr in range(2):
            t = consts.tile([P, D], BF16, name=f"ow{pr}")
            next_q().dma_start(out=t, in_=ow[P * pr:P * (pr + 1), :])
            ow_sb.append(t)

        # PE warm-up: dummy matmuls with no DMA deps keep the PE busy through
        # the preamble/DMA-ramp window so the HAM clock-gate opens before the
        # first real matmul arrives.
        warm = consts.tile([P, QCW], BF16, name="warm")
        nc.vector.memset(warm, 0.0)

        def pe_filler(count, tag):
            wps = spool.tile([P, 2, QCW], F32, name="s_ps")
            for i in range(count):
                nc.tensor.matmul(
                    wps[:, 0, 0:256], lhsT=warm[:, 0:P], rhs=warm[:, 0:256],
                    start=(i == 0), stop=(i == count - 1))

        pe_filler(30, "head")

        w_aug = {}
        ones_row = None
        if has_bias:
            ones_row = consts.tile([1, QCW], BF16, name="ones_row")
            nc.vector.memset(ones_row, 1.0)
            for key, nm in (("q", "wqA"), ("k", "wkA"), ("v", "wvA")):
                t = consts.tile([1, M_DIM], BF16, name=nm)
                nc.scalar.dma_start(out=t, in_=augs[nm])
                w_aug[key] = t

        QT = [qkv.tile([P, S], BF16, name=f"QT{pr}") for pr in range(2)]
        KT = [qkv.tile([P, S], BF16, name=f"KT{pr}") for pr in range(2)]
        CT = [qkv.tile([P, S], BF16, name=f"CT{pr}") for pr in range(2)]
        VA = [qkv.tile([P, HEADS, HD + 1], BF16, name=f"VA{t}") for t in range(NKT)]
        # softmax-denominator ones column, written once
        for t in range(NKT):
            nc.gpsimd.memset(VA[t][:, :, HD:HD + 1], 1.0)


        def emit_outproj(qc, cast_on_act=False):
            for mq in range(QCW // P):
                out_sb = outp.tile([P, D], BF16, name="out_sb")
                q0 = QCW * qc + P * mq
                for ne in range(2):
                    o_ps = opool.tile([P, QCW], F32, name="o_ps")
                    for pr2 in range(2):
                        nc.tensor.matmul(
                            o_ps,
                            lhsT=CT[pr2][:, q0:q0 + P],
                            rhs=ow_sb[pr2][:, QCW * ne:QCW * (ne + 1)],
                            start=(pr2 == 0), stop=(pr2 == 1))
                    # the tail out-projection runs after the last exp: the
                    # ACT engine is idle there, so casting one half on it
                    # halves the DVE-serial tail
                    if cast_on_act and ne == 1:
                        nc.scalar.copy(out_sb[:, QCW:], o_ps)
                    else:
                        nc.vector.tensor_copy(
                            out_sb[:, QCW * ne:QCW * (ne + 1)], o_ps)
                nc.sync.dma_start(out=out[q0:q0 + P, :], in_=out_sb)

        tri_bc = btri_sb.rearrange("p (a q) -> p a q", a=1).to_broadcast([P, 2, P])

        for n in range(NQC):
            # ---- stage n projections: q/k columns + v rows [512n, 512n+512) ----
            for key, dest in (("q", QT), ("k", KT)):
                for m in range(2):
                    ps = opool.tile([P, QCW], F32, name="o_ps")
                    for pc, (xtile, kc0, nkc) in enumerate(xt[key][n]):
                        last_pc = pc == len(xt[key][n]) - 1
                        for kc in range(nkc):
                            nc.tensor.matmul(
                                ps,
                                lhsT=w_sb[key][:, kc0 + kc, P * m:P * (m + 1)],
                                rhs=xtile[:, kc, :],
                                start=(kc0 + kc == 0),
                                stop=(not has_bias and last_pc and
                                      kc == nkc - 1))
                    if has_bias:
                        nc.tensor.matmul(
                            ps,
                            lhsT=w_aug[key][0:1, P * m:P * (m + 1)],
                            rhs=ones_row,
                            start=False, stop=True)
                    nc.vector.tensor_copy(
                        dest[m][:, QCW * n:QCW * (n + 1)], ps)
            for mv in range(4):
                m = 4 * n + mv
                ps = opool.tile([P, QCW], F32, name="o_ps")
                for pc, (xtile, kc0, nkc) in enumerate(xt["v"][n]):
                    last_pc = pc == len(xt["v"][n]) - 1
                    for kc in range(nkc):
                        nc.tensor.matmul(
                            ps[:, 0:M_DIM],
                            lhsT=xtile[:, kc, P * mv:P * (mv + 1)],
                            rhs=w_sb["v"][:, kc0 + kc, :],
                            start=(kc0 + kc == 0),
                            stop=(not has_bias and last_pc and kc == nkc - 1))
                if has_bias:
                    nc.tensor.matmul(
                        ps[:, 0:M_DIM],
                        lhsT=ones_row[0:1, 0:P],
                        rhs=w_aug["v"],
                        start=False, stop=True)
                nc.vector.tensor_copy(
                    VA[m][:, :, 0:HD],
                    ps[:, 0:M_DIM].rearrange("p (h d) -> p h d", h=HEADS))
            if n >= 2:
                emit_outproj(n - 2)

            # ---- stage n attention (q chunk n) ----
            qc = n
            for pr in range(2):
                nt = 4 * qc + 4 if mode == "causal" else NKT
                ctxs = [cpool.tile([HD + 1, QCW], F32, name="ctx_ps")
                        for _ in range(2)]
                queues = ([], [])

                def flush_ctx(j):
                    t0, p0, o0 = queues[j].pop(0)
                    nc.tensor.matmul(
                        ctxs[j][:, o0:],
                        lhsT=VA[t0][:, 2 * pr + j, :],
                        rhs=p0[:, j, o0:],
                        start=(t0 == 0), stop=(t0 == nt - 1),
                        skip_group_check=True)

                for t in range(nt):
                    o = max(0, P * t - QCW * qc) if mode == "causal" else 0
                    s_ps = spool.tile([P, 2, QCW], F32, name="s_ps")
                    for j in range(2):
                        nc.tensor.matmul(
                            s_ps[:, j, o:],
                            lhsT=KT[pr][HD * j:HD * (j + 1), P * t:P * (t + 1)],
                            rhs=QT[pr][HD * j:HD * (j + 1),
                                       QCW * qc + o:QCW * (qc + 1)],
                            start=True, stop=True,
                            tile_position=(HD * j, 0))
                    if mode == "generic":
                        bt = bpool.tile([P, QCW], F32, name="bt")
                        nc.sync.dma_start(
                            out=bt,
                            in_=bfull[P * t:P * (t + 1), QCW * qc:QCW * (qc + 1)])
                        nc.vector.tensor_add(
                            s_ps, s_ps,
                            bt.rearrange("p (a q) -> p a q", a=1)
                            .to_broadcast([P, 2, QCW]))
                    p_sb = ppool.tile([P, 2, QCW], BF16, name="p_sb")
                    nc.scalar.activation(
                        p_sb[:, :, o:], s_ps[:, :, o:], EXPF, scale=0.125)
                    if mode == "causal" and t >= 4 * qc:
                        # zero the upper triangle of the diagonal block
                        nc.vector.tensor_mul(
                            p_sb[:, :, o:o + P], p_sb[:, :, o:o + P], tri_bc)
                    for j in range(2):
                        queues[j].append((t, p_sb, o))
                    for j in range(2):
                        if len(queues[j]) > 2:
                            flush_ctx(j)
                if n == NQC - 1 and pr == 1:
                    # stage 3 is exp(ACT)-bound at the end: keep the PE busy
                    # (HAM warm) while the last exp tiles drain
                    pe_filler(20, "drain")
                for j in range(2):
                    while queues[j]:
                        flush_ctx(j)
                for j in range(2):
                    ctx_ps = ctxs[j]
                    l_sb = small.tile([1, QCW], F32, name="l_sb", bufs=3)
                    nc.vector.tensor_copy(l_sb, ctx_ps[HD:HD + 1, :])
                    r_sb = small.tile([1, QCW], F32, name="r_sb", bufs=3)
                    nc.vector.reciprocal_approx_fast(out=r_sb, in_=l_sb)
                    rbc = ppool.tile([HD, QCW], F32, name="rbc", bufs=2)
                    nc.gpsimd.partition_broadcast(out_ap=rbc, in_ap=r_sb)
                    nc.vector.tensor_mul(
                        CT[pr][HD * j:HD * (j + 1), QCW * qc:QCW * (qc + 1)],
                        ctx_ps[0:HD, :], rbc)

            if n == NQC - 1:
                # bridge the last norm-chain latency so the PE stays warm
                # (HAM) going into the final out-projection
                pe_filler(16, "tail")
        # out-projection of the last two q-chunks runs at the end: the last
        # q-chunk's attention is exp(ACT)-bound, and these matmuls are the
        # main PE work left to hide that
        emit_outproj(2, cast_on_act=True)
        emit_outproj(3, cast_on_act=True)

    if compile_:
        nc.compile()
    return nc


def _get_nc(mode, has_bias):
    key = (mode, has_bias)
    if key not in _NC_CACHE:
        _NC_CACHE[key] = build_nc(mode, has_bias=has_bias)
    return _NC_CACHE[key]


def _tri01():
    # 1.0 where q(col) >= k(row), else 0 (lower triangle incl diagonal kept)
    g = np.arange(P, dtype=np.int64)
    return np.where(g[None, :] >= g[:, None], 1.0, 0.0).astype(np.float32)


def host_prep(query, key, value, attn_mask, q_w, q_b, k_w, k_b, v_w, v_b, o_w, o_b):
    """Build (mode, in_maps) for the 8 cores."""
    mask = np.asarray(attn_mask).astype(bool)
    if np.array_equal(mask, np.triu(np.ones((S, S), bool), 1)):
        mode = "causal"
    elif not mask.any():
        mode = "nomask"
    else:
        mode = "generic"

    import ml_dtypes
    bf16 = ml_dtypes.bfloat16

    def prep_x(x):
        # [p, n, kc, c]; d = 128*kc + p, s = 512*n + c
        xtr = np.ascontiguousarray(np.asarray(x).T)
        a = xtr.reshape(KC, P, NQC, QCW).transpose(1, 2, 0, 3)
        return np.ascontiguousarray(a).astype(bf16).reshape(P, NQC * KC * QCW)

    xs = {}
    for b in range(2):
        xs[b] = (prep_x(np.asarray(query)[b]),
                 prep_x(np.asarray(key)[b]),
                 prep_x(np.asarray(value)[b]))

    tri = _tri01().astype(bf16)
    biasT = None
    if mode == "generic":
        biasT = np.ascontiguousarray(
            np.where(mask, np.float32(NEG), np.float32(0.0)).T)

    def prep_w(w, sl):
        wt = np.ascontiguousarray(np.asarray(w)[sl].T)  # [1024, 256]
        a = wt.reshape(KC, P, M_DIM).transpose(1, 0, 2)
        return np.ascontiguousarray(a).astype(bf16).reshape(P, KC * M_DIM)

    def prep_aug(bvec, sl):
        return np.ascontiguousarray(
            np.asarray(bvec)[sl][None, :]).astype(bf16)

    has_bias = any(np.asarray(b).any() for b in (q_b, k_b, v_b))
    in_maps = []
    for c in range(8):
        b, g = divmod(c, 4)
        sl = slice(M_DIM * g, M_DIM * (g + 1))
        m = {
            "xq16": xs[b][0], "xk16": xs[b][1], "xv16": xs[b][2],
            "wq16": prep_w(q_w, sl),
            "wk16": prep_w(k_w, sl),
            "wv16": prep_w(v_w, sl),
            "owT": np.ascontiguousarray(np.asarray(o_w)[:, sl].T).astype(bf16),
            "btri": tri,
        }
        if has_bias:
            m["wqA"] = prep_aug(q_b, sl)
            m["wkA"] = prep_aug(k_b, sl)
            m["wvA"] = prep_aug(v_b, sl)
        if mode == "generic":
            m["biasT"] = biasT
        in_maps.append(m)
    return mode, has_bias, in_maps


def kernel(**inputs) -> np.ndarray:
    global LAST_RESULTS
    from concourse.bass_utils import run_bass_kernel_spmd

    mode, has_bias, in_maps = host_prep(**inputs)
    nc = _get_nc(mode, has_bias)
    res = run_bass_kernel_spmd(nc, in_maps, core_ids=list(range(8)), trace=TRACE)
    LAST_RESULTS = res
    parts = [np.asarray(res.results[c]["out"]).astype(np.float32)
             for c in range(8)]
    o_b = np.asarray(inputs["o_b"]).astype(np.float32)
    out = np.stack([
        parts[0] + parts[1] + parts[2] + parts[3],
        parts[4] + parts[5] + parts[6] + parts[7],
    ], axis=0) + o_b[None, None, :]
    return out.astype(np.float32)


# revision 29
# speedup vs baseline: 1.0108x; 1.0059x over previous
"""Multi-head attention (B=2, S=2048, D=1024, H=16, causal mask) on 8 TRN2 cores.

Sharding: core c handles batch b = c//4 and 4 heads g = c%4 (dims 256g..256g+256
of the projection space).  Each core computes a partial output [S, D] (its 4
heads' contribution to the out-projection); the host sums the 4 partials per
batch and adds the output bias.

Device layout (per core) keeps the sequence axis on the SBUF free dimension:
  QT, KT  [256, 2048]  (head-dim on partitions, 2 head-pairs of 128)
  V_aug   16 tiles [128, 4, 65]  (seq on partitions; per head 64 dims + ones col)
  scores  S.T tiles [128 k, 512 q] per head; causal blocks above diagonal skipped
  exp     ScalarE, scale=1/8; causal diag block masked post-exp by a 0/1 bf16
          tri multiply on DVE (cheaper than the f32 PSUM bias add)
  ctx.T   [65, 512] PSUM per (head, q-chunk); row 64 = softmax denominator l
  norm    l copied to SBUF (reciprocal_approx_fast needs IEEE fp32 bits,
          so no direct PSUM read), reciprocal_approx_fast,
          partition_broadcast, DVE multiply
  out     ctxT (4 heads stacked, [256, 2048]) @ o_w slice -> [2048, 1024]

Perf structure:
  - all-bf16 compute: fp8 (even on the q/k path) fails the 2e-2 gate —
    multiplicative quantization noise propagates to the output at full
    relative strength (~5%); bf16 gives ~0.6%.
  - input DMAs are emitted in strict need-order, round-robined across the
    three DMA queues (sync/scalar HWDGE + gpsimd SWDGE): within a queue
    triggers process in order and the HW engines round-robin across
    queues, so aggregate HBM bandwidth always serves the next-needed
    tensors instead of fair-sharing with stage-2/3 prefetch.  Out DMA is
    one [128,1024] per 128-row block on sync.
  - the out-projection of q-chunks 2,3 is deferred to the end of the
    program: the last q-chunk's attention is exp(ACT)-throughput-bound,
    and those matmuls are the PE work that hides it.
"""

import numpy as np
from contextlib import ExitStack

import concourse.bacc as bacc
import concourse.bass as bass
import concourse.tile as tile
from concourse import mybir



P = 128
S = 2048
D = 1024
N_HEADS_TOT = 16
HEADS = 4            # per core
HD = 64
M_DIM = HEADS * HD   # 256
KC = 8               # embed-dim 128-chunks (bf16 v path)
QCW = 512            # q chunk width
NQC = S // QCW       # 4
NKT = S // P         # 16 k-tiles
F32 = mybir.dt.float32
BF16 = mybir.dt.bfloat16
EXPF = mybir.ActivationFunctionType.Exp
NEG = -1.0e9

TRACE = False
LAST_RESULTS = None
_NC_CACHE = {}


def build_nc(mode: str, compile_: bool = True,
             has_bias: bool = False) -> bass.Bass:
    """mode in {causal, nomask, generic}"""
    nc = bacc.Bacc("TRN2", target_bir_lowering=False, debug=False)
    # packed bf16 x: [p, stage, kc, c] ; d = 128*kc + p, s = 512*stage + c
    xin = {}
    for nm in ("q", "k", "v"):
        xin[nm] = nc.dram_tensor(f"x{nm}16", [P, NQC * KC * QCW], BF16,
                                 kind="ExternalInput").ap()
    # packed bf16 weights: [p, kc, m] ; row d = 128*kc + p
    win = {}
    for nm in ("q", "k", "v"):
        win[nm] = nc.dram_tensor(f"w{nm}16", [P, KC * M_DIM], BF16,
                                 kind="ExternalInput").ap()
    augs = {}
    if has_bias:
        for nm in ("wqA", "wkA", "wvA"):
            augs[nm] = nc.dram_tensor(nm, [1, M_DIM], BF16,
                                      kind="ExternalInput").ap()
    ow = nc.dram_tensor("owT", [M_DIM, D], BF16, kind="ExternalInput").ap()
    btri = nc.dram_tensor("btri", [P, P], BF16, kind="ExternalInput").ap()
    bfull = None
    if mode == "generic":
        bfull = nc.dram_tensor("biasT", [S, S], F32, kind="ExternalInput").ap()
    out = nc.dram_tensor("out", [S, D], BF16, kind="ExternalOutput").ap()

    with tile.TileContext(nc) as tc, ExitStack() as ctx:
        consts = ctx.enter_context(tc.tile_pool(name="consts", bufs=1))
        xpool = ctx.enter_context(tc.tile_pool(name="xpool", bufs=1))
        qkv = ctx.enter_context(tc.tile_pool(name="qkv", bufs=1))
        ppool = ctx.enter_context(tc.tile_pool(name="ppool", bufs=8))
        bpool = ctx.enter_context(tc.tile_pool(name="bpool", bufs=2))
        small = ctx.enter_context(tc.tile_pool(name="small", bufs=4))
        outp = ctx.enter_context(tc.tile_pool(name="outp", bufs=3))
        spool = ctx.enter_context(tc.tile_pool(name="spsum", bufs=2, space="PSUM"))
        opool = ctx.enter_context(tc.tile_pool(name="opsum", bufs=2, space="PSUM"))
        cpool = ctx.enter_context(tc.tile_pool(name="cpsum", bufs=2, space="PSUM"))

        # ---- weights + x tiles: contiguous DMAs, ordered by first need.
        # Strict need-order, round-robined across the three DMA queues:
        # within a queue triggers are processed in order, and the HW engines
        # round-robin across queues, so global need-order emission keeps the
        # aggregate HBM bandwidth on the next-needed tensors.
        rrq = [nc.sync, nc.scalar, nc.gpsimd]
        rr_i = [0]

        def next_q():
            e = rrq[rr_i[0] % 3]
            rr_i[0] += 1
            return e

        w_sb = {}
        for key in ("q", "k", "v"):
            t = consts.tile([P, KC, M_DIM], BF16, name=f"w16{key}")
            next_q().dma_start(out=t, in_=win[key].rearrange(
                "p (kc m) -> p kc m", kc=KC))
            w_sb[key] = t

        # xt[key][stage] = list of (tile, kc0, nkc) pieces
        xt = {"q": [], "k": [], "v": []}
        for key in ("q", "k", "v"):
            for n in range(NQC):
                xt[key].append([])

        def load_x(key, n, halves):
            dst = xt[key][n]
            nh = 2 if halves else 1
            nkc = KC // nh
            for h in range(nh):
                t = xpool.tile([P, nkc, QCW], BF16, name=f"x{key}{n}{h}")
                c0 = (KC * n + nkc * h) * QCW
                next_q().dma_start(
                    out=t,
                    in_=xin[key][:, c0:c0 + nkc * QCW].rearrange(
                        "p (kc c) -> p kc c", kc=nkc))
                dst.append((t, nkc * h, nkc))

        load_x("q", 0, True)
        load_x("k", 0, True)
        load_x("v", 0, True)
        btri_sb = consts.tile([P, P], BF16, name="btri_sb")
        next_q().dma_start(out=btri_sb, in_=btri)
        for n in range(1, NQC):
            for key in ("q", "k", "v"):
                load_x(key, n, False)
        ow_sb = []
        for pr in range(2):
            t = consts.tile([P, D], BF16, name=f"ow{pr}")
            next_q().dma_start(out=t, in_=ow[P * pr:P * (pr + 1), :])
            ow_sb.append(t)

        # PE warm-up: dummy matmuls with no DMA deps keep the PE busy through
        # the preamble/DMA-ramp window so the HAM clock-gate opens before the
        # first real matmul arrives.
        warm = consts.tile([P, QCW], BF16, name="warm")
        nc.vector.memset(warm, 0.0)

        def pe_filler(count, tag):
            wps = spool.tile([P, 2, QCW], F32, name="s_ps")
            for i in range(count):
                nc.tensor.matmul(
                    wps[:, 0, 0:256], lhsT=warm[:, 0:P], rhs=warm[:, 0:256],
                    start=(i == 0), stop=(i == count - 1))

        pe_filler(30, "head")

        w_aug = {}
        ones_row = None
        if has_bias:
            ones_row = consts.tile([1, QCW], BF16, name="ones_row")
            nc.vector.memset(ones_row, 1.0)
            for key, nm in (("q", "wqA"), ("k", "wkA"), ("v", "wvA")):
                t = consts.tile([1, M_DIM], BF16, name=nm)
                nc.scalar.dma_start(out=t, in_=augs[nm])
                w_aug[key] = t

        QT = [qkv.tile([P, S], BF16, name=f"QT{pr}") for pr in range(2)]
        KT = [qkv.tile([P, S], BF16, name=f"KT{pr}") for pr in range(2)]
        CT = [qkv.tile([P, S], BF16, name=f"CT{pr}") for pr in range(2)]
        VA = [qkv.tile([P, HEADS, HD + 1], BF16, name=f"VA{t}") for t in range(NKT)]
        # softmax-denominator ones column, written once
        for t in range(NKT):
            nc.gpsimd.memset(VA[t][:, :, HD:HD + 1], 1.0)


        def emit_outproj(qc, cast_on_act=False):
            for mq in range(QCW // P):
                out_sb = outp.tile([P, D], BF16, name="out_sb")
                q0 = QCW * qc + P * mq
                for ne in range(2):
                    o_ps = opool.tile([P, QCW], F32, name="o_ps")
                    for pr2 in range(2):
                        nc.tensor.matmul(
                            o_ps,
                            lhsT=CT[pr2][:, q0:q0 + P],
                            rhs=ow_sb[pr2][:, QCW * ne:QCW * (ne + 1)],
                            start=(pr2 == 0), stop=(pr2 == 1))
                    # the tail out-projection runs after the last exp: the
                    # ACT engine is idle there, so casting one half on it
                    # halves the DVE-serial tail
                    if cast_on_act and ne == 1:
                        nc.scalar.copy(out_sb[:, QCW:], o_ps)
                    else:
                        nc.vector.tensor_copy(
                            out_sb[:, QCW * ne:QCW * (ne + 1)], o_ps)
                nc.sync.dma_start(out=out[q0:q0 + P, :], in_=out_sb)

        tri_bc = btri_sb.rearrange("p (a q) -> p a q", a=1).to_broadcast([P, 2, P])

        for n in range(NQC):
            # ---- stage n projections: q/k columns + v rows [512n, 512n+512) ----
            for key, dest in (("q", QT), ("k", KT)):
                for m in range(2):
                    ps = opool.tile([P, QCW], F32, name="o_ps")
                    for pc, (xtile, kc0, nkc) in enumerate(xt[key][n]):
                        last_pc = pc == len(xt[key][n]) - 1
                        for kc in range(nkc):
                            nc.tensor.matmul(
                                ps,
                                lhsT=w_sb[key][:, kc0 + kc, P * m:P * (m + 1)],
                                rhs=xtile[:, kc, :],
                                start=(kc0 + kc == 0),
                                stop=(not has_bias and last_pc and
                                      kc == nkc - 1))
                    if has_bias:
                        nc.tensor.matmul(
                            ps,
                            lhsT=w_aug[key][0:1, P * m:P * (m + 1)],
                            rhs=ones_row,
                            start=False, stop=True)
                    nc.vector.tensor_copy(
                        dest[m][:, QCW * n:QCW * (n + 1)], ps)
            for mv in range(4):
                m = 4 * n + mv
                ps = opool.tile([P, QCW], F32, name="o_ps")
                for pc, (xtile, kc0, nkc) in enumerate(xt["v"][n]):
                    last_pc = pc == len(xt["v"][n]) - 1
                    for kc in range(nkc):
                        nc.tensor.matmul(
                            ps[:, 0:M_DIM],
                            lhsT=xtile[:, kc, P * mv:P * (mv + 1)],
                            rhs=w_sb["v"][:, kc0 + kc, :],
                            start=(kc0 + kc == 0),
                            stop=(not has_bias and last_pc and kc == nkc - 1))
                if has_bias:
                    nc.tensor.matmul(
                        ps[:, 0:M_DIM],
                        lhsT=ones_row[0:1, 0:P],
                        rhs=w_aug["v"],
                        start=False, stop=True)
                nc.vector.tensor_copy(
                    VA[m][:, :, 0:HD],
                    ps[:, 0:M_DIM].rearrange("p (h d) -> p h d", h=HEADS))
            if n >= 2:
                emit_outproj(n - 2)

            # ---- stage n attention (q chunk n) ----
            qc = n
            for pr in range(2):
                nt = 4 * qc + 4 if mode == "causal" else NKT
                ctxs = [cpool.tile([HD + 1, QCW], F32, name="ctx_ps")
                        for _ in range(2)]
                queues = ([], [])

                def flush_ctx(j):
                    t0, p0, o0 = queues[j].pop(0)
                    nc.tensor.matmul(
                        ctxs[j][:, o0:],
                        lhsT=VA[t0][:, 2 * pr + j, :],
                        rhs=p0[:, j, o0:],
                        start=(t0 == 0), stop=(t0 == nt - 1),
                        skip_group_check=True)

                for t in range(nt):
                    o = max(0, P * t - QCW * qc) if mode == "causal" else 0
                    s_ps = spool.tile([P, 2, QCW], F32, name="s_ps")
                    for j in range(2):
                        nc.tensor.matmul(
                            s_ps[:, j, o:],
                            lhsT=KT[pr][HD * j:HD * (j + 1), P * t:P * (t + 1)],
                            rhs=QT[pr][HD * j:HD * (j + 1),
                                       QCW * qc + o:QCW * (qc + 1)],
                            start=True, stop=True,
                            tile_position=(HD * j, 0))
                    if mode == "generic":
                        bt = bpool.tile([P, QCW], F32, name="bt")
                        nc.sync.dma_start(
                            out=bt,
                            in_=bfull[P * t:P * (t + 1), QCW * qc:QCW * (qc + 1)])
                        nc.vector.tensor_add(
                            s_ps, s_ps,
                            bt.rearrange("p (a q) -> p a q", a=1)
                            .to_broadcast([P, 2, QCW]))
                    p_sb = ppool.tile([P, 2, QCW], BF16, name="p_sb")
                    nc.scalar.activation(
                        p_sb[:, :, o:], s_ps[:, :, o:], EXPF, scale=0.125)
                    if mode == "causal" and t >= 4 * qc:
                        # zero the upper triangle of the diagonal block
                        nc.vector.tensor_mul(
                            p_sb[:, :, o:o + P], p_sb[:, :, o:o + P], tri_bc)
                    for j in range(2):
                        queues[j].append((t, p_sb, o))
                    for j in range(2):
                        if len(queues[j]) > 2:
                            flush_ctx(j)
                for j in range(2):
                    while queues[j]:
                        flush_ctx(j)
                for j in range(2):
                    ctx_ps = ctxs[j]
                    l_sb = small.tile([1, QCW], F32, name="l_sb", bufs=3)
                    nc.vector.tensor_copy(l_sb, ctx_ps[HD:HD + 1, :])
                    r_sb = small.tile([1, QCW], F32, name="r_sb", bufs=3)
                    nc.vector.reciprocal_approx_fast(out=r_sb, in_=l_sb)
                    rbc = ppool.tile([HD, QCW], F32, name="rbc", bufs=2)
                    nc.gpsimd.partition_broadcast(out_ap=rbc, in_ap=r_sb)
                    nc.vector.tensor_mul(
                        CT[pr][HD * j:HD * (j + 1), QCW * qc:QCW * (qc + 1)],
                        ctx_ps[0:HD, :], rbc)

        # out-projection of the last two q-chunks runs at the end: the last
        # q-chunk's attention is exp(ACT)-bound, and these matmuls are the
        # main PE work left to hide that
        emit_outproj(2, cast_on_act=True)
        emit_outproj(3, cast_on_act=True)

    if compile_:
        nc.compile()
    return nc


def _get_nc(mode, has_bias):
    key = (mode, has_bias)
    if key not in _NC_CACHE:
        _NC_CACHE[key] = build_nc(mode, has_bias=has_bias)
    return _NC_CACHE[key]


def _tri01():
    # 1.0 where q(col) >= k(row), else 0 (lower triangle incl diagonal kept)
    g = np.arange(P, dtype=np.int64)
    return np.where(g[None, :] >= g[:, None], 1.0, 0.0).astype(np.float32)


def host_prep(query, key, value, attn_mask, q_w, q_b, k_w, k_b, v_w, v_b, o_w, o_b):
    """Build (mode, in_maps) for the 8 cores."""
    mask = np.asarray(attn_mask).astype(bool)
    if np.array_equal(mask, np.triu(np.ones((S, S), bool), 1)):
        mode = "causal"
    elif not mask.any():
        mode = "nomask"
    else:
        mode = "generic"

    import ml_dtypes
    bf16 = ml_dtypes.bfloat16

    def prep_x(x):
        # [p, n, kc, c]; d = 128*kc + p, s = 512*n + c
        xtr = np.ascontiguousarray(np.asarray(x).T)
        a = xtr.reshape(KC, P, NQC, QCW).transpose(1, 2, 0, 3)
        return np.ascontiguousarray(a).astype(bf16).reshape(P, NQC * KC * QCW)

    xs = {}
    for b in range(2):
        xs[b] = (prep_x(np.asarray(query)[b]),
                 prep_x(np.asarray(key)[b]),
                 prep_x(np.asarray(value)[b]))

    tri = _tri01().astype(bf16)
    biasT = None
    if mode == "generic":
        biasT = np.ascontiguousarray(
            np.where(mask, np.float32(NEG), np.float32(0.0)).T)

    def prep_w(w, sl):
        wt = np.ascontiguousarray(np.asarray(w)[sl].T)  # [1024, 256]
        a = wt.reshape(KC, P, M_DIM).transpose(1, 0, 2)
        return np.ascontiguousarray(a).astype(bf16).reshape(P, KC * M_DIM)

    def prep_aug(bvec, sl):
        return np.ascontiguousarray(
            np.asarray(bvec)[sl][None, :]).astype(bf16)

    has_bias = any(np.asarray(b).any() for b in (q_b, k_b, v_b))
    in_maps = []
    for c in range(8):
        b, g = divmod(c, 4)
        sl = slice(M_DIM * g, M_DIM * (g + 1))
        m = {
            "xq16": xs[b][0], "xk16": xs[b][1], "xv16": xs[b][2],
            "wq16": prep_w(q_w, sl),
            "wk16": prep_w(k_w, sl),
            "wv16": prep_w(v_w, sl),
            "owT": np.ascontiguousarray(np.asarray(o_w)[:, sl].T).astype(bf16),
            "btri": tri,
        }
        if has_bias:
            m["wqA"] = prep_aug(q_b, sl)
            m["wkA"] = prep_aug(k_b, sl)
            m["wvA"] = prep_aug(v_b, sl)
        if mode == "generic":
            m["biasT"] = biasT
        in_maps.append(m)
    return mode, has_bias, in_maps


def kernel(**inputs) -> np.ndarray:
    global LAST_RESULTS
    from concourse.bass_utils import run_bass_kernel_spmd

    mode, has_bias, in_maps = host_prep(**inputs)
    nc = _get_nc(mode, has_bias)
    res = run_bass_kernel_spmd(nc, in_maps, core_ids=list(range(8)), trace=TRACE)
    LAST_RESULTS = res
    parts = [np.asarray(res.results[c]["out"]).astype(np.float32)
             for c in range(8)]
    o_b = np.asarray(inputs["o_b"]).astype(np.float32)
    out = np.stack([
        parts[0] + parts[1] + parts[2] + parts[3],
        parts[4] + parts[5] + parts[6] + parts[7],
    ], axis=0) + o_b[None, None, :]
    return out.astype(np.float32)


# revision 30
# speedup vs baseline: 1.0202x; 1.0093x over previous
"""Multi-head attention (B=2, S=2048, D=1024, H=16, causal mask) on 8 TRN2 cores.

Sharding: core c handles batch b = c//4 and 4 heads g = c%4 (dims 256g..256g+256
of the projection space).  Each core computes a partial output [S, D] (its 4
heads' contribution to the out-projection); the host sums the 4 partials per
batch and adds the output bias.

Device layout (per core) keeps the sequence axis on the SBUF free dimension:
  QT, KT  [256, 2048]  (head-dim on partitions, 2 head-pairs of 128)
  V_aug   16 tiles [128, 4, 65]  (seq on partitions; per head 64 dims + ones col)
  scores  S.T tiles [128 k, 512 q] per head; causal blocks above diagonal skipped
  exp     ScalarE, scale=1/8; causal diag block masked post-exp by a 0/1 bf16
          tri multiply on DVE (cheaper than the f32 PSUM bias add)
  ctx.T   [65, 512] PSUM per (head, q-chunk); row 64 = softmax denominator l
  norm    l copied to SBUF (reciprocal_approx_fast needs IEEE fp32 bits,
          so no direct PSUM read), reciprocal_approx_fast,
          partition_broadcast, DVE multiply
  out     ctxT (4 heads stacked, [256, 2048]) @ o_w slice -> [2048, 1024]

Perf structure:
  - all-bf16 compute: fp8 (even on the q/k path) fails the 2e-2 gate —
    multiplicative quantization noise propagates to the output at full
    relative strength (~5%); bf16 gives ~0.6%.
  - input DMAs are emitted in strict need-order, round-robined across the
    three DMA queues (sync/scalar HWDGE + gpsimd SWDGE): within a queue
    triggers process in order and the HW engines round-robin across
    queues, so aggregate HBM bandwidth always serves the next-needed
    tensors instead of fair-sharing with stage-2/3 prefetch.  Out DMA is
    one [128,1024] per 128-row block on sync.
  - the out-projection of q-chunks 2,3 is deferred to the end of the
    program: the last q-chunk's attention is exp(ACT)-throughput-bound,
    and those matmuls are the PE work that hides it.
"""

import numpy as np
from contextlib import ExitStack

import concourse.bacc as bacc
import concourse.bass as bass
import concourse.tile as tile
from concourse import mybir



P = 128
S = 2048
D = 1024
N_HEADS_TOT = 16
HEADS = 4            # per core
HD = 64
M_DIM = HEADS * HD   # 256
KC = 8               # embed-dim 128-chunks (bf16 v path)
QCW = 512            # q chunk width
NQC = S // QCW       # 4
NKT = S // P         # 16 k-tiles
F32 = mybir.dt.float32
BF16 = mybir.dt.bfloat16
EXPF = mybir.ActivationFunctionType.Exp
NEG = -1.0e9

TRACE = False
LAST_RESULTS = None
_NC_CACHE = {}


def build_nc(mode: str, compile_: bool = True,
             has_bias: bool = False) -> bass.Bass:
    """mode in {causal, nomask, generic}"""
    nc = bacc.Bacc("TRN2", target_bir_lowering=False, debug=False)
    # packed bf16 x: [p, stage, kc, c] ; d = 128*kc + p, s = 512*stage + c
    xin = {}
    for nm in ("q", "k", "v"):
        xin[nm] = nc.dram_tensor(f"x{nm}16", [P, NQC * KC * QCW], BF16,
                                 kind="ExternalInput").ap()
    # packed bf16 weights: [p, kc, m] ; row d = 128*kc + p
    win = {}
    for nm in ("q", "k", "v"):
        win[nm] = nc.dram_tensor(f"w{nm}16", [P, KC * M_DIM], BF16,
                                 kind="ExternalInput").ap()
    augs = {}
    if has_bias:
        for nm in ("wqA", "wkA", "wvA"):
            augs[nm] = nc.dram_tensor(nm, [1, M_DIM], BF16,
                                      kind="ExternalInput").ap()
    ow = nc.dram_tensor("owT", [M_DIM, D], BF16, kind="ExternalInput").ap()
    btri = nc.dram_tensor("btri", [P, P], BF16, kind="ExternalInput").ap()
    bfull = None
    if mode == "generic":
        bfull = nc.dram_tensor("biasT", [S, S], F32, kind="ExternalInput").ap()
    out = nc.dram_tensor("out", [S, D], BF16, kind="ExternalOutput").ap()

    with tile.TileContext(nc) as tc, ExitStack() as ctx:
        consts = ctx.enter_context(tc.tile_pool(name="consts", bufs=1))
        xpool = ctx.enter_context(tc.tile_pool(name="xpool", bufs=1))
        qkv = ctx.enter_context(tc.tile_pool(name="qkv", bufs=1))
        ppool = ctx.enter_context(tc.tile_pool(name="ppool", bufs=8))
        bpool = ctx.enter_context(tc.tile_pool(name="bpool", bufs=2))
        small = ctx.enter_context(tc.tile_pool(name="small", bufs=4))
        outp = ctx.enter_context(tc.tile_pool(name="outp", bufs=3))
        spool = ctx.enter_context(tc.tile_pool(name="spsum", bufs=2, space="PSUM"))
        opool = ctx.enter_context(tc.tile_pool(name="opsum", bufs=2, space="PSUM"))
        cpool = ctx.enter_context(tc.tile_pool(name="cpsum", bufs=2, space="PSUM"))

        # ---- weights + x tiles: contiguous DMAs, ordered by first need.
        # Strict need-order, round-robined across the three DMA queues:
        # within a queue triggers are processed in order, and the HW engines
        # round-robin across queues, so global need-order emission keeps the
        # aggregate HBM bandwidth on the next-needed tensors.
        rrq = [nc.sync, nc.scalar, nc.gpsimd]
        rr_i = [0]

        def next_q():
            e = rrq[rr_i[0] % 3]
            rr_i[0] += 1
            return e

        w_sb = {}
        for key in ("q", "k", "v"):
            t = consts.tile([P, KC, M_DIM], BF16, name=f"w16{key}")
            next_q().dma_start(out=t, in_=win[key].rearrange(
                "p (kc m) -> p kc m", kc=KC))
            w_sb[key] = t

        # xt[key][stage] = list of (tile, kc0, nkc) pieces
        xt = {"q": [], "k": [], "v": []}
        for key in ("q", "k", "v"):
            for n in range(NQC):
                xt[key].append([])

        def load_x(key, n, halves):
            dst = xt[key][n]
            nh = 2 if halves else 1
            nkc = KC // nh
            for h in range(nh):
                t = xpool.tile([P, nkc, QCW], BF16, name=f"x{key}{n}{h}")
                c0 = (KC * n + nkc * h) * QCW
                next_q().dma_start(
                    out=t,
                    in_=xin[key][:, c0:c0 + nkc * QCW].rearrange(
                        "p (kc c) -> p kc c", kc=nkc))
                dst.append((t, nkc * h, nkc))

        load_x("q", 0, True)
        load_x("k", 0, True)
        load_x("v", 0, True)
        btri_sb = consts.tile([P, P], BF16, name="btri_sb")
        next_q().dma_start(out=btri_sb, in_=btri)
        for n in range(1, NQC):
            for key in ("q", "k", "v"):
                load_x(key, n, False)
        ow_sb = []
        for pr in range(2):
            t = consts.tile([P, D], BF16, name=f"ow{pr}")
            next_q().dma_start(out=t, in_=ow[P * pr:P * (pr + 1), :])
            ow_sb.append(t)

        # PE warm-up: dummy matmuls with no DMA deps keep the PE busy through
        # the preamble/DMA-ramp window so the HAM clock-gate opens before the
        # first real matmul arrives.
        warm = consts.tile([P, QCW], BF16, name="warm")
        nc.vector.memset(warm, 0.0)

        def pe_filler(count, tag):
            wps = spool.tile([P, 2, QCW], F32, name="s_ps")
            for i in range(count):
                nc.tensor.matmul(
                    wps[:, 0, 0:256], lhsT=warm[:, 0:P], rhs=warm[:, 0:256],
                    start=(i == 0), stop=(i == count - 1))

        pe_filler(30, "head")

        w_aug = {}
        ones_row = None
        if has_bias:
            ones_row = consts.tile([1, QCW], BF16, name="ones_row")
            nc.vector.memset(ones_row, 1.0)
            for key, nm in (("q", "wqA"), ("k", "wkA"), ("v", "wvA")):
                t = consts.tile([1, M_DIM], BF16, name=nm)
                nc.scalar.dma_start(out=t, in_=augs[nm])
                w_aug[key] = t

        QT = [qkv.tile([P, S], BF16, name=f"QT{pr}") for pr in range(2)]
        KT = [qkv.tile([P, S], BF16, name=f"KT{pr}") for pr in range(2)]
        CT = [qkv.tile([P, S], BF16, name=f"CT{pr}") for pr in range(2)]
        VA = [qkv.tile([P, HEADS, HD + 1], BF16, name=f"VA{t}") for t in range(NKT)]
        # softmax-denominator ones column, written once
        for t in range(NKT):
            nc.gpsimd.memset(VA[t][:, :, HD:HD + 1], 1.0)


        def emit_outproj(qc, cast_on_act=False):
            for mq in range(QCW // P):
                out_sb = outp.tile([P, D], BF16, name="out_sb")
                q0 = QCW * qc + P * mq
                for ne in range(2):
                    o_ps = opool.tile([P, QCW], F32, name="o_ps")
                    for pr2 in range(2):
                        nc.tensor.matmul(
                            o_ps,
                            lhsT=CT[pr2][:, q0:q0 + P],
                            rhs=ow_sb[pr2][:, QCW * ne:QCW * (ne + 1)],
                            start=(pr2 == 0), stop=(pr2 == 1))
                    # the tail out-projection runs after the last exp: the
                    # ACT engine is idle there, so casting one half on it
                    # halves the DVE-serial tail
                    if cast_on_act and ne == 1:
                        nc.scalar.copy(out_sb[:, QCW:], o_ps)
                    else:
                        nc.vector.tensor_copy(
                            out_sb[:, QCW * ne:QCW * (ne + 1)], o_ps)
                nc.sync.dma_start(out=out[q0:q0 + P, :], in_=out_sb)

        tri_bc = btri_sb.rearrange("p (a q) -> p a q", a=1).to_broadcast([P, 2, P])

        for n in range(NQC):
            # ---- stage n projections: q/k columns + v rows [512n, 512n+512) ----
            for key, dest in (("q", QT), ("k", KT)):
                for m in range(2):
                    ps = opool.tile([P, QCW], F32, name="o_ps")
                    for pc, (xtile, kc0, nkc) in enumerate(xt[key][n]):
                        last_pc = pc == len(xt[key][n]) - 1
                        for kc in range(nkc):
                            nc.tensor.matmul(
                                ps,
                                lhsT=w_sb[key][:, kc0 + kc, P * m:P * (m + 1)],
                                rhs=xtile[:, kc, :],
                                start=(kc0 + kc == 0),
                                stop=(not has_bias and last_pc and
                                      kc == nkc - 1))
                    if has_bias:
                        nc.tensor.matmul(
                            ps,
                            lhsT=w_aug[key][0:1, P * m:P * (m + 1)],
                            rhs=ones_row,
                            start=False, stop=True)
                    nc.vector.tensor_copy(
                        dest[m][:, QCW * n:QCW * (n + 1)], ps)
            for mv in range(4):
                m = 4 * n + mv
                ps = opool.tile([P, QCW], F32, name="o_ps")
                for pc, (xtile, kc0, nkc) in enumerate(xt["v"][n]):
                    last_pc = pc == len(xt["v"][n]) - 1
                    for kc in range(nkc):
                        nc.tensor.matmul(
                            ps[:, 0:M_DIM],
                            lhsT=xtile[:, kc, P * mv:P * (mv + 1)],
                            rhs=w_sb["v"][:, kc0 + kc, :],
                            start=(kc0 + kc == 0),
                            stop=(not has_bias and last_pc and kc == nkc - 1))
                if has_bias:
                    nc.tensor.matmul(
                        ps[:, 0:M_DIM],
                        lhsT=ones_row[0:1, 0:P],
                        rhs=w_aug["v"],
                        start=False, stop=True)
                nc.vector.tensor_copy(
                    VA[m][:, :, 0:HD],
                    ps[:, 0:M_DIM].rearrange("p (h d) -> p h d", h=HEADS))
            if n >= 2:
                emit_outproj(n - 2)

            # ---- stage n attention (q chunk n) ----
            qc = n
            for pr in range(2):
                nt = 4 * qc + 4 if mode == "causal" else NKT
                ctxs = [cpool.tile([HD + 1, QCW], F32, name="ctx_ps")
                        for _ in range(2)]
                queues = ([], [])

                def flush_ctx(j):
                    t0, p0, o0 = queues[j].pop(0)
                    nc.tensor.matmul(
                        ctxs[j][:, o0:],
                        lhsT=VA[t0][:, 2 * pr + j, :],
                        rhs=p0[:, j, o0:],
                        start=(t0 == 0), stop=(t0 == nt - 1),
                        skip_group_check=True)

                for t in range(nt):
                    o = max(0, P * t - QCW * qc) if mode == "causal" else 0
                    s_ps = spool.tile([P, 2, QCW], F32, name="s_ps")
                    for j in range(2):
                        nc.tensor.matmul(
                            s_ps[:, j, o:],
                            lhsT=KT[pr][HD * j:HD * (j + 1), P * t:P * (t + 1)],
                            rhs=QT[pr][HD * j:HD * (j + 1),
                                       QCW * qc + o:QCW * (qc + 1)],
                            start=True, stop=True,
                            tile_position=(HD * j, 0))
                    if mode == "generic":
                        bt = bpool.tile([P, QCW], F32, name="bt")
                        nc.sync.dma_start(
                            out=bt,
                            in_=bfull[P * t:P * (t + 1), QCW * qc:QCW * (qc + 1)])
                        nc.vector.tensor_add(
                            s_ps, s_ps,
                            bt.rearrange("p (a q) -> p a q", a=1)
                            .to_broadcast([P, 2, QCW]))
                    p_sb = ppool.tile([P, 2, QCW], BF16, name="p_sb")
                    nc.scalar.activation(
                        p_sb[:, :, o:], s_ps[:, :, o:], EXPF, scale=0.125)
                    if mode == "causal" and t >= 4 * qc:
                        # zero the upper triangle of the diagonal block
                        nc.vector.tensor_mul(
                            p_sb[:, :, o:o + P], p_sb[:, :, o:o + P], tri_bc)
                    for j in range(2):
                        queues[j].append((t, p_sb, o))
                    for j in range(2):
                        if len(queues[j]) > 2:
                            flush_ctx(j)
                for j in range(2):
                    while queues[j]:
                        flush_ctx(j)
                for j in range(2):
                    ctx_ps = ctxs[j]
                    l_sb = small.tile([1, QCW], F32, name="l_sb", bufs=3)
                    nc.vector.tensor_copy(l_sb, ctx_ps[HD:HD + 1, :])
                    r_sb = small.tile([1, QCW], F32, name="r_sb", bufs=3)
                    nc.vector.reciprocal_approx_fast(out=r_sb, in_=l_sb)
                    rbc = ppool.tile([HD, QCW], F32, name="rbc", bufs=2)
                    nc.gpsimd.partition_broadcast(out_ap=rbc, in_ap=r_sb)
                    nc.vector.tensor_mul(
                        CT[pr][HD * j:HD * (j + 1), QCW * qc:QCW * (qc + 1)],
                        ctx_ps[0:HD, :], rbc)

        # out-projection of the last two q-chunks runs at the end: the last
        # q-chunk's attention is exp(ACT)-bound, and these matmuls are the
        # main PE work left to hide that.  The ACT-bound stretch drops PE
        # duty below the HAM threshold (half clock); a >=3.4us dense burst
        # reopens the clock gate so the tail matmuls run at full rate.
        pe_filler(34, "rewarm")
        emit_outproj(2, cast_on_act=True)
        emit_outproj(3, cast_on_act=True)

    if compile_:
        nc.compile()
    return nc


def _get_nc(mode, has_bias):
    key = (mode, has_bias)
    if key not in _NC_CACHE:
        _NC_CACHE[key] = build_nc(mode, has_bias=has_bias)
    return _NC_CACHE[key]


def _tri01():
    # 1.0 where q(col) >= k(row), else 0 (lower triangle incl diagonal kept)
    g = np.arange(P, dtype=np.int64)
    return np.where(g[None, :] >= g[:, None], 1.0, 0.0).astype(np.float32)


def host_prep(query, key, value, attn_mask, q_w, q_b, k_w, k_b, v_w, v_b, o_w, o_b):
    """Build (mode, in_maps) for the 8 cores."""
    mask = np.asarray(attn_mask).astype(bool)
    if np.array_equal(mask, np.triu(np.ones((S, S), bool), 1)):
        mode = "causal"
    elif not mask.any():
        mode = "nomask"
    else:
        mode = "generic"

    import ml_dtypes
    bf16 = ml_dtypes.bfloat16

    def prep_x(x):
        # [p, n, kc, c]; d = 128*kc + p, s = 512*n + c
        xtr = np.ascontiguousarray(np.asarray(x).T)
        a = xtr.reshape(KC, P, NQC, QCW).transpose(1, 2, 0, 3)
        return np.ascontiguousarray(a).astype(bf16).reshape(P, NQC * KC * QCW)

    xs = {}
    for b in range(2):
        xs[b] = (prep_x(np.asarray(query)[b]),
                 prep_x(np.asarray(key)[b]),
                 prep_x(np.asarray(value)[b]))

    tri = _tri01().astype(bf16)
    biasT = None
    if mode == "generic":
        biasT = np.ascontiguousarray(
            np.where(mask, np.float32(NEG), np.float32(0.0)).T)

    def prep_w(w, sl):
        wt = np.ascontiguousarray(np.asarray(w)[sl].T)  # [1024, 256]
        a = wt.reshape(KC, P, M_DIM).transpose(1, 0, 2)
        return np.ascontiguousarray(a).astype(bf16).reshape(P, KC * M_DIM)

    def prep_aug(bvec, sl):
        return np.ascontiguousarray(
            np.asarray(bvec)[sl][None, :]).astype(bf16)

    has_bias = any(np.asarray(b).any() for b in (q_b, k_b, v_b))
    in_maps = []
    for c in range(8):
        b, g = divmod(c, 4)
        sl = slice(M_DIM * g, M_DIM * (g + 1))
        m = {
            "xq16": xs[b][0], "xk16": xs[b][1], "xv16": xs[b][2],
            "wq16": prep_w(q_w, sl),
            "wk16": prep_w(k_w, sl),
            "wv16": prep_w(v_w, sl),
            "owT": np.ascontiguousarray(np.asarray(o_w)[:, sl].T).astype(bf16),
            "btri": tri,
        }
        if has_bias:
            m["wqA"] = prep_aug(q_b, sl)
            m["wkA"] = prep_aug(k_b, sl)
            m["wvA"] = prep_aug(v_b, sl)
        if mode == "generic":
            m["biasT"] = biasT
        in_maps.append(m)
    return mode, has_bias, in_maps


def kernel(**inputs) -> np.ndarray:
    global LAST_RESULTS
    from concourse.bass_utils import run_bass_kernel_spmd

    mode, has_bias, in_maps = host_prep(**inputs)
    nc = _get_nc(mode, has_bias)
    res = run_bass_kernel_spmd(nc, in_maps, core_ids=list(range(8)), trace=TRACE)
    LAST_RESULTS = res
    parts = [np.asarray(res.results[c]["out"]).astype(np.float32)
             for c in range(8)]
    o_b = np.asarray(inputs["o_b"]).astype(np.float32)
    out = np.stack([
        parts[0] + parts[1] + parts[2] + parts[3],
        parts[4] + parts[5] + parts[6] + parts[7],
    ], axis=0) + o_b[None, None, :]
    return out.astype(np.float32)
